# revision 50
# baseline (speedup 1.0000x reference)
"""Vocab-sharded AdaptiveSoftmax (log_softmax loss head) on 8 TRN2 NeuronCores.

Reference, for x:[2,512,1024] (T=1024 tokens, H=1024):
  head  = x @ W_head.T          -> cols 0:20000 raw logits + 2 cluster logits
  tail1 = cl0 + log_softmax(x @ W_proj1.T @ W_tail1.T)   (40000 vocab)
  tail2 = cl1 + log_softmax(x @ W_proj2.T @ W_tail2.T)   (140000 vocab)
  out   = concat([head[:, :20000], tail1, tail2], -1)

Sharding: vocab dim of head/tail weights split 8 ways (2500/5000/17500 rows
per core, pre-transposed, x32-scaled + fp8e4-cast on host); x replicated.
log_softmax normalizers = AllReduce(add) of per-token exp-sums (the data
distribution keeps |logits| < ~3, so no max-subtraction is needed).

All matmuls run fp8e4 DoubleRow (2 contraction rows per pass); the 1/32
de-scale rides free on ACT `scale` / DVE tensor_scalar. Per 2048-col PSUM
seg: ACT computes exp+accum-sum from PSUM (discard output), and the seg
drains raw logits to a bf16 stage via DVE tensor_scalar_mul (most segs) or
ACT Identity (last 2-3 segs, balancing the engines). Finalize = one DVE
4x-mode tensor_scalar_add of (cl - ln S) over the whole stage. Only
Exp/Ln/Identity are used and the act-table chooser is pinned to the one
set containing all three, so the table loads once.

Phase order P -> T1 -> T2 -> H: tail1's two 4-tile AllReduces resolve
under tail2's first tiles, tail2's per-tile AllReduces resolve 2 tiles
later (finalize issued mid-tile t+2), and the PE-heavy head phase covers
tail2's last finalizes. A dummy warm-up AllReduce during P absorbs the
first-collective latency. Engine split: PE fp8-DR matmuls; DVE drains +
bias adds; ACT exps + a minority of drains; collectives on TOPSP; outputs
and weights on the Sync HWDGE queue, cc staging + g-loads on GpSimd's.
"""

import sys

import numpy as np

if "/opt/trn_rl_repo" not in sys.path:
    sys.path.insert(0, "/opt/trn_rl_repo")

P = 128
T = 1024          # tokens (2*512)
NT = T // P       # 8 token tiles
H = 1024
KO_H = H // P     # 8
VH = 2500         # head vocab shard
VHp = 2512        # padded to %16 for DoubleRow rhs step
V1 = 5000         # tail1 vocab shard
V1p = 5008
V2 = 17500        # tail2 vocab shard
V2p = 17504
E1, E2 = 512, 256
KO_1, KO_2 = E1 // P, E2 // P
C = 512           # matmul free-dim sub-block (psum bank / DR moving limit)
N_CORES = 8
VOUT = VH + V1 + V2   # 25000 per-core out cols
WSC = 32.0        # host-side weight scale into fp8 normal range
ISC = 1.0 / WSC
SEG = 1536        # tail psum tile width (3 f32 banks)

T2SEGS = [(i * SEG, SEG) for i in range(11)] + [(11 * SEG, V2p - 11 * SEG)]
T1SEGS = [(i * SEG, SEG) for i in range(3)] + [(3 * SEG, V1p - 3 * SEG)]
HSUBS = [(0, 512), (512, 512), (1024, 512), (1536, 512), (2048, VHp - 2048)]
NA2 = 8                    # tail2 staged segs per tile; rest deferred
AW2 = NA2 * SEG            # 12288 staged cols
BW2 = V2 - AW2             # 5212 deferred cols (bias fused on recompute)
DEFSEGS = T2SEGS[NA2:]

_CACHE = {}


def _pin_act_tables():
    """Make Exp/Ln resolve only to natural_log_exp_and_others so the ACT
    table loads once instead of thrashing between per-function sets.
    Set ids stay valid: we only shrink the fn sets used by the chooser."""
    import concourse.hw_specs as hw_specs
    import concourse.mybir as mybir
    tabs = hw_specs.get_activation_tables("gen3")  # functools.cached dict
    for name, fns in tabs.items():
        if name != "natural_log_exp_and_others":
            fns.discard(mybir.ActivationFunctionType.Exp)
            fns.discard(mybir.ActivationFunctionType.Ln)


def _build():
    import concourse.bacc as bacc
    import concourse.mybir as mybir
    import concourse.tile as tile
    from contextlib import ExitStack

    _pin_act_tables()

    f8 = mybir.dt.float8e4
    bf16 = mybir.dt.bfloat16
    f32 = mybir.dt.float32
    Exp = mybir.ActivationFunctionType.Exp
    Ident = mybir.ActivationFunctionType.Identity
    Ln = mybir.ActivationFunctionType.Ln
    DR = mybir.MatmulPerfMode.DoubleRow
    AX = mybir.AxisListType.X

    nc = bacc.Bacc("TRN2", target_bir_lowering=False, debug=False,
                   num_devices=N_CORES)

    xT_d = nc.declare_dram_parameter("xT", [P, KO_H, T], f8, False)
    whead_d = nc.declare_dram_parameter("wheadT", [P, KO_H, VHp], f8, False)
    wcl_d = nc.declare_dram_parameter("wclT", [P, KO_H, 2], f8, False)
    wp1_d = nc.declare_dram_parameter("wp1T", [P, KO_H, E1], f8, False)
    wp2_d = nc.declare_dram_parameter("wp2T", [P, KO_H, E2], f8, False)
    wt1_d = nc.declare_dram_parameter("wt1T", [P, KO_1, V1p], f8, False)
    wt2_d = nc.declare_dram_parameter("wt2T", [P, KO_2, V2p], f8, False)
    out_d = nc.declare_dram_parameter("out", [T, VOUT], bf16, True)

    out_r = out_d.ap().rearrange("(t p) v -> p t v", p=P)
    rg = [list(range(N_CORES))]

    with tile.TileContext(nc) as tc:
        with ExitStack() as root:
            pers = root.enter_context(tc.tile_pool(name="pers", bufs=1))
            psum = root.enter_context(
                tc.tile_pool(name="psum", bufs=2, space="PSUM"))
            psumS = root.enter_context(
                tc.tile_pool(name="psumS", bufs=2, space="PSUM"))
            dram = root.enter_context(
                tc.tile_pool(name="dram", bufs=1, space="DRAM"))
            scratch = root.enter_context(tc.tile_pool(name="scratch", bufs=2))

            # persistent small tiles
            p2T = pers.tile([P, KO_2, T], f8, name="p2T")
            cl = pers.tile([P, NT, 2], f32, name="cl")
            s1acc = pers.tile([P, NT, 4], f32, name="s1acc")
            s2acc = pers.tile([P, NT, 12], f32, name="s2acc")
            s1 = pers.tile([P, NT], f32, name="s1")
            s2 = pers.tile([P, NT], f32, name="s2")
            g1 = pers.tile([P, NT], f32, name="g1")
            g2 = pers.tile([P, NT], f32, name="g2")
            b1 = pers.tile([P, NT], f32, name="b1")
            b2 = pers.tile([P, NT], f32, name="b2")
            sc2 = pers.tile([P, NT], f32, name="sc2")
            wrm = pers.tile([P, 1], f32, name="wrm")
            exb = scratch.tile([P, SEG], f8, tag="exb", bufs=1)

            ccw_in = dram.tile([P, 1], f32, name="ccw_in")
            ccw_out = dram.tile([P, 1], f32, name="ccw_out",
                                addr_space="Shared")
            cc1_in = [dram.tile([P, 4], f32, name=f"cc1_in{i}")
                      for i in range(2)]
            cc1_out = [dram.tile([P, 4], f32, name=f"cc1_out{i}",
                                 addr_space="Shared") for i in range(2)]
            cc2_in = [dram.tile([P, 2], f32, name=f"cc2_in{b}")
                      for b in range(NT // 2)]
            cc2_out = [dram.tile([P, 2], f32, name=f"cc2_out{b}",
                                 addr_space="Shared") for b in range(NT // 2)]

            def mm_seg(ps, w, lhsT_sb, kop, t, rhs_sb, voff):
                """Accumulate [128 tokens, w] logits (x32 scale) into psum ps
                for token tile t via DoubleRow fp8: kop k-pairs, rhs columns
                voff:voff+w."""
                for kk in range(kop):
                    for sub in range(0, w, C):
                        sw = min(C, w - sub)
                        nc.tensor.matmul(
                            ps[:, sub:sub + sw],
                            lhsT_sb[:, 2 * kk:2 * kk + 2, t * P:(t + 1) * P],
                            rhs_sb[:, 2 * kk:2 * kk + 2,
                                   voff + sub:voff + sub + sw],
                            start=(kk == 0), stop=(kk == kop - 1),
                            perf_mode=DR)

            # ================= Phase P =================
            xT_pool = tc.alloc_tile_pool(name="xTp", bufs=1, side="right")
            xT = xT_pool.tile([P, KO_H, T], f8, name="xT")
            whead_pool = tc.alloc_tile_pool(name="wheadp", bufs=1,
                                            side="right")
            whead = whead_pool.tile([P, KO_H, VHp], f8, name="whead")
            p1T_pool = tc.alloc_tile_pool(name="p1Tp", bufs=1)
            p1Tl = p1T_pool.tile([P, KO_1, T], f8, name="p1Tl")
            wt1_pool = tc.alloc_tile_pool(name="wt1p", bufs=1)
            wt1 = wt1_pool.tile([P, KO_1, V1p], f8, name="wt1")
            wp_pool = tc.alloc_tile_pool(name="wpp", bufs=1)
            wp1 = wp_pool.tile([P, KO_H, E1], f8, name="wp1")
            wp2 = wp_pool.tile([P, KO_H, E2], f8, name="wp2")
            wcl = wp_pool.tile([P, KO_H, 2], f8, name="wcl")

            nc.sync.dma_start(wp1[:], wp1_d[:])
            nc.sync.dma_start(wcl[:], wcl_d[:])
            nc.sync.dma_start(wp2[:], wp2_d[:])
            nc.sync.dma_start(xT[:], xT_d[:])
            nc.sync.dma_start(wt1[:], wt1_d[:])

            # warm up the collectives pipe (first AR pays ~30us extra)
            nc.vector.memset(wrm[:], 1.0)
            nc.gpsimd.dma_start(ccw_in[:], wrm[:])
            nc.gpsimd.collective_compute(
                "AllReduce", mybir.AluOpType.add, replica_groups=rg,
                ins=[ccw_in[:].opt()], outs=[ccw_out[:].opt()])

            for proj_sb, wp_sb, ko in ((p1Tl, wp1, KO_1), (p2T, wp2, KO_2)):
                for e in range(ko):
                    for th in range(2):
                        ps = psumS.tile([P, 512], f32, tag="hs")
                        for kk in range(KO_H // 2):
                            nc.tensor.matmul(
                                ps[:],
                                wp_sb[:, 2 * kk:2 * kk + 2,
                                      e * P:(e + 1) * P],
                                xT[:, 2 * kk:2 * kk + 2,
                                   th * 512:(th + 1) * 512],
                                start=(kk == 0), stop=(kk == KO_H // 2 - 1),
                                perf_mode=DR)
                        nc.scalar.activation(
                            proj_sb[:, e, th * 512:(th + 1) * 512],
                            ps[:], Ident, scale=ISC)
            for t in range(NT):
                ps = psumS.tile([P, 512], f32, tag="hs")
                for kk in range(KO_H // 2):
                    nc.tensor.matmul(
                        ps[:, :2],
                        xT[:, 2 * kk:2 * kk + 2, t * P:(t + 1) * P],
                        wcl[:, 2 * kk:2 * kk + 2, :],
                        start=(kk == 0), stop=(kk == KO_H // 2 - 1),
                        perf_mode=DR)
                nc.vector.tensor_scalar_mul(cl[:, t, :], ps[:, :2], ISC)
            wp_pool.release()

            wt2_pool = tc.alloc_tile_pool(name="wt2p", bufs=1, side="right")
            wt2 = wt2_pool.tile([P, KO_2, V2p], f8, name="wt2")
            for off, w in T2SEGS:
                nc.sync.dma_start(wt2[:, :, off:off + w],
                                  wt2_d[:, :, off:off + w])
            nc.sync.dma_start(whead[:], whead_d[:])
            obuf_pool = [tc.alloc_tile_pool(name="obuf", bufs=4,
                                            side="right")]
            ob2_pool = [tc.alloc_tile_pool(name="ob2", bufs=1,
                                           side="right")]

            # ================= Phase T1 =================
            # tiles 4-7 pool allocated first: it is released later (LIFO)
            sp1L = tc.alloc_tile_pool(name="sp1L", bufs=4, side="right")
            sp1E = tc.alloc_tile_pool(name="sp1E", bufs=4, side="right")
            stg1 = {}

            def t1_compute(t):
                pool = sp1E if t < 4 else sp1L
                stg = pool.tile([P, V1], bf16, name=f"stg1_{t}", tag="s1")
                stg1[t] = stg
                for si, (off, w) in enumerate(T1SEGS):
                    if w > 512:
                        ps = psum.tile([P, SEG], f32, tag="mm")
                    else:
                        ps = psumS.tile([P, 512], f32, tag="hs")
                    mm_seg(ps, w, p1Tl, KO_1 // 2, t, wt1, off)
                    wt = min(w, V1 - off)
                    nc.scalar.activation(
                        exb[:, :wt], ps[:, :wt], Exp, scale=ISC,
                        accum_out=s1acc[:, t, si:si + 1])
                    if si < 3:
                        nc.vector.tensor_scalar_mul(
                            stg[:, off:off + wt], ps[:, :wt], ISC)
                    else:
                        nc.scalar.activation(
                            stg[:, off:off + wt], ps[:, :wt], Ident,
                            scale=ISC)
                nc.vector.reduce_sum(s1[:, t:t + 1], s1acc[:, t, :], axis=AX)

            def t1_ar(i):  # i = batch 0 (tiles 0-3) or 1 (tiles 4-7)
                nc.gpsimd.dma_start(cc1_in[i][:], s1[:, 4 * i:4 * i + 4])
                nc.gpsimd.collective_compute(
                    "AllReduce", mybir.AluOpType.add, replica_groups=rg,
                    ins=[cc1_in[i][:].opt()], outs=[cc1_out[i][:].opt()])

            def t1_bias(i):
                nc.gpsimd.dma_start(g1[:, 4 * i:4 * i + 4], cc1_out[i][:])
                lng = scratch.tile([P, 4], f32, tag="lng4")
                nc.scalar.activation(lng[:], g1[:, 4 * i:4 * i + 4], Ln)
                nc.vector.tensor_sub(out=b1[:, 4 * i:4 * i + 4],
                                     in0=cl[:, 4 * i:4 * i + 4, 0],
                                     in1=lng[:])

            def t1_finalize(t):
                nc.vector.tensor_scalar_add(stg1[t][:], stg1[t][:],
                                            b1[:, t:t + 1])
                nc.sync.dma_start(out_r[:, t, VH:VH + V1], stg1[t][:])

            for t in range(4):
                t1_compute(t)
            t1_ar(0)
            t1_compute(4)
            t1_compute(5)
            t1_compute(6)
            t1_bias(0)
            t1_finalize(0)
            t1_finalize(1)
            t1_compute(7)
            t1_ar(1)
            t1_finalize(2)
            t1_finalize(3)
            sp1E.release()
            wt1_pool.release()
            p1T_pool.release()

            # ================= Phase T2 (head tiles interleaved) ========
            headout_pool = tc.alloc_tile_pool(name="headoutp", bufs=2)
            sp2a = tc.alloc_tile_pool(name="sp2a", bufs=2)
            sp2b = [None]   # allocated after sp1L release
            stg2 = {}

            def t2_compute(t, mids=()):
                mids = dict(mids)
                pool = sp2a if (t // 2) % 2 == 0 else sp2b[0]
                stg = pool.tile([P, AW2], bf16, name=f"stg2_{t}", tag="s2")
                stg2[t] = stg
                for si, (off, w) in enumerate(T2SEGS):
                    for m in mids.get(si, ()):
                        m()
                    ps = psum.tile([P, SEG], f32, tag="mm")
                    mm_seg(ps, w, p2T, KO_2 // 2, t, wt2, off)
                    wt = min(w, V2 - off)
                    if si < NA2:
                        # A region: exp for the sum (discard), raw-logit
                        # drain to the bf16 stage
                        nc.scalar.activation(
                            exb[:, :wt], ps[:, :wt], Exp, scale=ISC,
                            accum_out=s2acc[:, t, si:si + 1])
                        nc.vector.tensor_scalar_mul(
                            stg[:, off:off + wt], ps[:, :wt], ISC)
                    else:
                        # B region: keep the exp values (fp8) — finalize
                        # recovers logit+bias via Ln(exp(b) * exp(a))
                        if si == NA2:
                            obufs[t] = obuf_pool[0].tile(
                                [P, BW2], f8, tag="ob", name=f"ob{t}")
                        nc.scalar.activation(
                            obufs[t][:, off - AW2:off - AW2 + wt],
                            ps[:, :wt], Exp, scale=ISC,
                            accum_out=s2acc[:, t, si:si + 1])
                nc.vector.reduce_sum(s2[:, t:t + 1], s2acc[:, t, :], axis=AX)
                if t % 2:
                    b = t // 2
                    nc.gpsimd.dma_start(cc2_in[b][:], s2[:, t - 1:t + 1])
                    nc.gpsimd.collective_compute(
                        "AllReduce", mybir.AluOpType.add, replica_groups=rg,
                        ins=[cc2_in[b][:].opt()], outs=[cc2_out[b][:].opt()])

            def t2_bias(b):
                nc.gpsimd.dma_start(g2[:, 2 * b:2 * b + 2], cc2_out[b][:])
                lng = scratch.tile([P, 2], f32, tag="lng2")
                nc.scalar.activation(lng[:], g2[:, 2 * b:2 * b + 2], Ln)
                nc.vector.tensor_sub(out=b2[:, 2 * b:2 * b + 2],
                                     in0=cl[:, 2 * b:2 * b + 2, 1],
                                     in1=lng[:])
                nc.scalar.activation(sc2[:, 2 * b:2 * b + 2],
                                     b2[:, 2 * b:2 * b + 2], Exp)



            def t2_finA(t):
                # staged region: add bias, ship
                nc.vector.tensor_scalar_add(stg2[t][:], stg2[t][:],
                                            b2[:, t:t + 1])
                nc.sync.dma_start(out_r[:, t, VH + V1:VH + V1 + AW2],
                                  stg2[t][:])

            obufs = {}

            def t2_finB(t):
                # B region: logit+bias = Ln(exp(b) * exp(logit))
                ob2 = ob2_pool[0].tile([P, BW2], bf16, tag="ob2",
                                       name=f"ob2_{t}")
                nc.scalar.activation(ob2[:], obufs[t][:, :BW2], Ln,
                                     scale=sc2[:, t:t + 1])
                nc.sync.dma_start(out_r[:, t, VH + V1 + AW2:VOUT], ob2[:])

            # ============ Phase H (instruction-level interleave) ========
            # head subs run on their own 1-bank psum stream, spread into
            # the tail2 seg loop so the PE always has an independent
            # matmul to fill drain-bound gaps.
            hos = {}

            def h_sub(t, k):
                if k == 0:
                    hos[t] = headout_pool.tile([P, VHp], bf16, tag="ho",
                                               name=f"ho{t}")
                ho = hos[t]
                off, w = HSUBS[k]
                ps = psumS.tile([P, 512], f32, tag="hs")
                for kk in range(KO_H // 2):
                    nc.tensor.matmul(
                        ps[:, :w],
                        xT[:, 2 * kk:2 * kk + 2, t * P:(t + 1) * P],
                        whead[:, 2 * kk:2 * kk + 2, off:off + w],
                        start=(kk == 0), stop=(kk == KO_H // 2 - 1),
                        perf_mode=DR)
                if k < 3:
                    nc.vector.tensor_scalar_mul(
                        ho[:, off:off + w], ps[:, :w], ISC)
                else:
                    nc.scalar.activation(ho[:, off:off + w], ps[:, :w],
                                         Ident, scale=ISC)
                if k == 4:
                    nc.sync.dma_start(out_r[:, t, 0:VH], ho[:, :VH])

            def h_tile(t):
                for k in range(5):
                    h_sub(t, k)

            def h_mids(t):
                return {2: (lambda: h_sub(t, 0),),
                        4: (lambda: h_sub(t, 1),),
                        6: (lambda: h_sub(t, 2),),
                        8: (lambda: h_sub(t, 3),),
                        10: (lambda: h_sub(t, 4),)}

            def fin_mids(b):
                # finalize pair {2b, 2b+1} spread through tile 2b+3
                t0, t1x = 2 * b, 2 * b + 1
                return {3: (lambda: t2_bias(b), lambda: t2_finA(t0)),
                        5: (lambda: t2_finB(t0),),
                        7: (lambda: t2_finA(t1x),),
                        9: (lambda: t2_finB(t1x),)}

            h_tile(0)
            t2_compute(0, mids={5: (lambda: t1_bias(1),
                                    lambda: t1_finalize(4),
                                    lambda: t1_finalize(5),
                                    lambda: t1_finalize(6),
                                    lambda: t1_finalize(7))})
            sp1L.release()
            sp2b[0] = tc.alloc_tile_pool(name="sp2b", bufs=2)
            t2_compute(1, mids=h_mids(1))
            t2_compute(2, mids=h_mids(2))
            t2_compute(3, mids={**h_mids(3), **fin_mids(0)})
            t2_compute(4, mids=h_mids(4))
            t2_compute(5, mids=fin_mids(1))
            t2_compute(6, mids=h_mids(5))
            t2_compute(7, mids=fin_mids(2))
            # tail: h6+h7 cover AR{6,7}; fin DMAs front-loaded, def
            # recomputes keep the PE fed while the out queue drains
            h_tile(6)
            h_tile(7)
            t2_bias(3)
            t2_finA(6)
            t2_finB(6)
            t2_finA(7)
            t2_finB(7)

            # unwind pools (stack order per side)
            sp2b[0].release()
            sp2a.release()
            headout_pool.release()
            ob2_pool[0].release()
            obuf_pool[0].release()
            wt2_pool.release()
            whead_pool.release()
            xT_pool.release()

    nc.compile()
    return nc


def _get_nc():
    if "nc" not in _CACHE:
        _CACHE["nc"] = _build()
    return _CACHE["nc"]


def _prep_inputs(x, W_head, W_proj1, W_tail1, W_proj2, W_tail2):
    import concourse.mybir as mybir
    f8np = mybir.dt.np(mybir.dt.float8e4)

    def kxn8(w, pad_to=None, scale=WSC):
        # [N, K] weight -> [128, K//128, Np] fp8 (K on partitions), x scale
        n, k = w.shape
        a = np.ascontiguousarray(
            w.T.reshape(k // P, P, n).transpose(1, 0, 2)) * scale
        if pad_to is not None and pad_to != n:
            a = np.concatenate(
                [a, np.zeros((P, k // P, pad_to - n), np.float32)], axis=2)
        return a.astype(f8np)

    x2 = np.asarray(x, np.float32).reshape(T, H)
    xT = np.ascontiguousarray(
        x2.T.reshape(KO_H, P, T).transpose(1, 0, 2)).astype(f8np)
    wcl = kxn8(W_head[20000:20002])
    wp1 = kxn8(W_proj1)
    wp2 = kxn8(W_proj2)

    in_maps = []
    for i in range(N_CORES):
        in_maps.append({
            "xT": xT,
            "wheadT": kxn8(W_head[i * VH:(i + 1) * VH], VHp),
            "wclT": wcl,
            "wp1T": wp1,
            "wp2T": wp2,
            "wt1T": kxn8(W_tail1[i * V1:(i + 1) * V1], V1p),
            "wt2T": kxn8(W_tail2[i * V2:(i + 1) * V2], V2p),
        })
    return in_maps


def _assemble(outs):
    final = np.empty((T, 200000), dtype=np.float32)
    for i in range(N_CORES):
        o = np.asarray(outs[i]["out"])
        final[:, i * VH:(i + 1) * VH] = o[:, :VH]
        final[:, 20000 + i * V1:20000 + (i + 1) * V1] = o[:, VH:VH + V1]
        final[:, 60000 + i * V2:60000 + (i + 1) * V2] = o[:, VH + V1:]
    return final.reshape(2, 512, 200000)


def _run(inputs, trace=False, tmpdir=None):
    from concourse import bass_utils
    nc = _get_nc()
    in_maps = _prep_inputs(**inputs)
    res = bass_utils.run_bass_kernel_spmd(
        nc, in_maps, core_ids=list(range(N_CORES)), trace=trace,
        tmpdir=tmpdir)
    return _assemble(res.results), res


def kernel(**inputs):
    inputs = {k: np.asarray(v) for k, v in inputs.items()}
    out, _ = _run(inputs, trace=False)
    return out


# revision 52
# speedup vs baseline: 1.0116x; 1.0116x over previous
"""Vocab-sharded AdaptiveSoftmax (log_softmax loss head) on 8 TRN2 NeuronCores.

Reference, for x:[2,512,1024] (T=1024 tokens, H=1024):
  head  = x @ W_head.T          -> cols 0:20000 raw logits + 2 cluster logits
  tail1 = cl0 + log_softmax(x @ W_proj1.T @ W_tail1.T)   (40000 vocab)
  tail2 = cl1 + log_softmax(x @ W_proj2.T @ W_tail2.T)   (140000 vocab)
  out   = concat([head[:, :20000], tail1, tail2], -1)

Sharding: vocab dim of head/tail weights split 8 ways (2500/5000/17500 rows
per core, pre-transposed, x32-scaled + fp8e4-cast on host); x replicated.
log_softmax normalizers = AllReduce(add) of per-token exp-sums (the data
distribution keeps |logits| < ~3, so no max-subtraction is needed).

All matmuls run fp8e4 DoubleRow (2 contraction rows per pass); the 1/32
de-scale rides free on ACT `scale` / DVE tensor_scalar. Per 2048-col PSUM
seg: ACT computes exp+accum-sum from PSUM (discard output), and the seg
drains raw logits to a bf16 stage via DVE tensor_scalar_mul (most segs) or
ACT Identity (last 2-3 segs, balancing the engines). Finalize = one DVE
4x-mode tensor_scalar_add of (cl - ln S) over the whole stage. Only
Exp/Ln/Identity are used and the act-table chooser is pinned to the one
set containing all three, so the table loads once.

Phase order P -> T1 -> T2 -> H: tail1's two 4-tile AllReduces resolve
under tail2's first tiles, tail2's per-tile AllReduces resolve 2 tiles
later (finalize issued mid-tile t+2), and the PE-heavy head phase covers
tail2's last finalizes. A dummy warm-up AllReduce during P absorbs the
first-collective latency. Engine split: PE fp8-DR matmuls; DVE drains +
bias adds; ACT exps + a minority of drains; collectives on TOPSP; outputs
and weights on the Sync HWDGE queue, cc staging + g-loads on GpSimd's.
"""

import sys

import numpy as np

if "/opt/trn_rl_repo" not in sys.path:
    sys.path.insert(0, "/opt/trn_rl_repo")

P = 128
T = 1024          # tokens (2*512)
NT = T // P       # 8 token tiles
H = 1024
KO_H = H // P     # 8
VH = 2500         # head vocab shard
VHp = 2512        # padded to %16 for DoubleRow rhs step
V1 = 5000         # tail1 vocab shard
V1p = 5008
V2 = 17500        # tail2 vocab shard
V2p = 17504
E1, E2 = 512, 256
KO_1, KO_2 = E1 // P, E2 // P
C = 512           # matmul free-dim sub-block (psum bank / DR moving limit)
N_CORES = 8
VOUT = VH + V1 + V2   # 25000 per-core out cols
WSC = 32.0        # host-side weight scale into fp8 normal range
ISC = 1.0 / WSC
SEG = 1536        # tail psum tile width (3 f32 banks)

T2SEGS = [(i * SEG, SEG) for i in range(11)] + [(11 * SEG, V2p - 11 * SEG)]
T1SEGS = [(i * SEG, SEG) for i in range(3)] + [(3 * SEG, V1p - 3 * SEG)]
HSUBS = [(0, 512), (512, 512), (1024, 512), (1536, 512), (2048, VHp - 2048)]
NA2 = 8                    # tail2 staged segs per tile; rest deferred
AW2 = NA2 * SEG            # 12288 staged cols
BW2 = V2 - AW2             # 5212 deferred cols (bias fused on recompute)
DEFSEGS = T2SEGS[NA2:]

_CACHE = {}


def _pin_act_tables():
    """Make Exp/Ln resolve only to natural_log_exp_and_others so the ACT
    table loads once instead of thrashing between per-function sets.
    Set ids stay valid: we only shrink the fn sets used by the chooser."""
    import concourse.hw_specs as hw_specs
    import concourse.mybir as mybir
    tabs = hw_specs.get_activation_tables("gen3")  # functools.cached dict
    for name, fns in tabs.items():
        if name != "natural_log_exp_and_others":
            fns.discard(mybir.ActivationFunctionType.Exp)
            fns.discard(mybir.ActivationFunctionType.Ln)


def _build():
    import concourse.bacc as bacc
    import concourse.mybir as mybir
    import concourse.tile as tile
    from contextlib import ExitStack

    _pin_act_tables()

    f8 = mybir.dt.float8e4
    bf16 = mybir.dt.bfloat16
    f32 = mybir.dt.float32
    Exp = mybir.ActivationFunctionType.Exp
    Ident = mybir.ActivationFunctionType.Identity
    Ln = mybir.ActivationFunctionType.Ln
    DR = mybir.MatmulPerfMode.DoubleRow
    AX = mybir.AxisListType.X

    nc = bacc.Bacc("TRN2", target_bir_lowering=False, debug=False,
                   num_devices=N_CORES)

    xT_d = nc.declare_dram_parameter("xT", [P, KO_H, T], f8, False)
    whead_d = nc.declare_dram_parameter("wheadT", [P, KO_H, VHp], f8, False)
    wcl_d = nc.declare_dram_parameter("wclT", [P, KO_H, 2], f8, False)
    wp1_d = nc.declare_dram_parameter("wp1T", [P, KO_H, E1], f8, False)
    wp2_d = nc.declare_dram_parameter("wp2T", [P, KO_H, E2], f8, False)
    wt1_d = nc.declare_dram_parameter("wt1T", [P, KO_1, V1p], f8, False)
    wt2_d = nc.declare_dram_parameter("wt2T", [P, KO_2, V2p], f8, False)
    out_d = nc.declare_dram_parameter("out", [T, VOUT], bf16, True)

    out_r = out_d.ap().rearrange("(t p) v -> p t v", p=P)
    rg = [list(range(N_CORES))]

    with tile.TileContext(nc) as tc:
        with ExitStack() as root:
            pers = root.enter_context(tc.tile_pool(name="pers", bufs=1))
            psum = root.enter_context(
                tc.tile_pool(name="psum", bufs=2, space="PSUM"))
            psumS = root.enter_context(
                tc.tile_pool(name="psumS", bufs=2, space="PSUM"))
            dram = root.enter_context(
                tc.tile_pool(name="dram", bufs=1, space="DRAM"))
            scratch = root.enter_context(tc.tile_pool(name="scratch", bufs=2))

            # persistent small tiles
            p2T = pers.tile([P, KO_2, T], f8, name="p2T")
            cl = pers.tile([P, NT, 2], f32, name="cl")
            s1acc = pers.tile([P, NT, 4], f32, name="s1acc")
            s2acc = pers.tile([P, NT, 12], f32, name="s2acc")
            s1 = pers.tile([P, NT], f32, name="s1")
            s2 = pers.tile([P, NT], f32, name="s2")
            g1 = pers.tile([P, NT], f32, name="g1")
            g2 = pers.tile([P, NT], f32, name="g2")
            b1 = pers.tile([P, NT], f32, name="b1")
            b2 = pers.tile([P, NT], f32, name="b2")
            wrm = pers.tile([P, 1], f32, name="wrm")
            exb = scratch.tile([P, SEG], f8, tag="exb", bufs=1)

            ccw_in = dram.tile([P, 1], f32, name="ccw_in")
            ccw_out = dram.tile([P, 1], f32, name="ccw_out",
                                addr_space="Shared")
            cc1_in = [dram.tile([P, 4], f32, name=f"cc1_in{i}")
                      for i in range(2)]
            cc1_out = [dram.tile([P, 4], f32, name=f"cc1_out{i}",
                                 addr_space="Shared") for i in range(2)]
            cc2_in = [dram.tile([P, 2], f32, name=f"cc2_in{b}")
                      for b in range(NT // 2)]
            cc2_out = [dram.tile([P, 2], f32, name=f"cc2_out{b}",
                                 addr_space="Shared") for b in range(NT // 2)]

            def mm_seg(ps, w, lhsT_sb, kop, t, rhs_sb, voff):
                """Accumulate [128 tokens, w] logits (x32 scale) into psum ps
                for token tile t via DoubleRow fp8: kop k-pairs, rhs columns
                voff:voff+w."""
                for kk in range(kop):
                    for sub in range(0, w, C):
                        sw = min(C, w - sub)
                        nc.tensor.matmul(
                            ps[:, sub:sub + sw],
                            lhsT_sb[:, 2 * kk:2 * kk + 2, t * P:(t + 1) * P],
                            rhs_sb[:, 2 * kk:2 * kk + 2,
                                   voff + sub:voff + sub + sw],
                            start=(kk == 0), stop=(kk == kop - 1),
                            perf_mode=DR)

            # ================= Phase P =================
            xT_pool = tc.alloc_tile_pool(name="xTp", bufs=1, side="right")
            xT = xT_pool.tile([P, KO_H, T], f8, name="xT")
            whead_pool = tc.alloc_tile_pool(name="wheadp", bufs=1,
                                            side="right")
            whead = whead_pool.tile([P, KO_H, VHp], f8, name="whead")
            p1T_pool = tc.alloc_tile_pool(name="p1Tp", bufs=1)
            p1Tl = p1T_pool.tile([P, KO_1, T], f8, name="p1Tl")
            wt1_pool = tc.alloc_tile_pool(name="wt1p", bufs=1)
            wt1 = wt1_pool.tile([P, KO_1, V1p], f8, name="wt1")
            wp_pool = tc.alloc_tile_pool(name="wpp", bufs=1)
            wp1 = wp_pool.tile([P, KO_H, E1], f8, name="wp1")
            wp2 = wp_pool.tile([P, KO_H, E2], f8, name="wp2")
            wcl = wp_pool.tile([P, KO_H, 2], f8, name="wcl")

            nc.sync.dma_start(wcl[:], wcl_d[:])
            nc.sync.dma_start(xT[:], xT_d[:])
            nc.sync.dma_start(wp1[:], wp1_d[:])
            nc.sync.dma_start(wp2[:], wp2_d[:])
            nc.sync.dma_start(wt1[:], wt1_d[:])

            # warm up the collectives pipe (first AR pays ~30us extra)
            nc.vector.memset(wrm[:], 1.0)
            nc.gpsimd.dma_start(ccw_in[:], wrm[:])
            nc.gpsimd.collective_compute(
                "AllReduce", mybir.AluOpType.add, replica_groups=rg,
                ins=[ccw_in[:].opt()], outs=[ccw_out[:].opt()])

            for t in range(NT):
                ps = psumS.tile([P, 512], f32, tag="hs")
                for kk in range(KO_H // 2):
                    nc.tensor.matmul(
                        ps[:, :2],
                        xT[:, 2 * kk:2 * kk + 2, t * P:(t + 1) * P],
                        wcl[:, 2 * kk:2 * kk + 2, :],
                        start=(kk == 0), stop=(kk == KO_H // 2 - 1),
                        perf_mode=DR)
                nc.vector.tensor_scalar_mul(cl[:, t, :], ps[:, :2], ISC)
            for proj_sb, wp_sb, ko in ((p1Tl, wp1, KO_1), (p2T, wp2, KO_2)):
                for e in range(ko):
                    for th in range(2):
                        ps = psumS.tile([P, 512], f32, tag="hs")
                        for kk in range(KO_H // 2):
                            nc.tensor.matmul(
                                ps[:],
                                wp_sb[:, 2 * kk:2 * kk + 2,
                                      e * P:(e + 1) * P],
                                xT[:, 2 * kk:2 * kk + 2,
                                   th * 512:(th + 1) * 512],
                                start=(kk == 0), stop=(kk == KO_H // 2 - 1),
                                perf_mode=DR)
                        nc.scalar.activation(
                            proj_sb[:, e, th * 512:(th + 1) * 512],
                            ps[:], Ident, scale=ISC)
            wp_pool.release()

            wt2_pool = tc.alloc_tile_pool(name="wt2p", bufs=1, side="right")
            wt2 = wt2_pool.tile([P, KO_2, V2p], f8, name="wt2")
            for off, w in T2SEGS:
                nc.sync.dma_start(wt2[:, :, off:off + w],
                                  wt2_d[:, :, off:off + w])
            nc.sync.dma_start(whead[:], whead_d[:])

            # ================= Phase T1 =================
            # tiles 4-7 pool allocated first: it is released later (LIFO)
            sp1L = tc.alloc_tile_pool(name="sp1L", bufs=4, side="right")
            sp1E = tc.alloc_tile_pool(name="sp1E", bufs=4, side="right")
            stg1 = {}

            def t1_compute(t):
                pool = sp1E if t < 4 else sp1L
                stg = pool.tile([P, V1], bf16, name=f"stg1_{t}", tag="s1")
                stg1[t] = stg
                for si, (off, w) in enumerate(T1SEGS):
                    if w > 512:
                        ps = psum.tile([P, SEG], f32, tag="mm")
                    else:
                        ps = psumS.tile([P, 512], f32, tag="hs")
                    mm_seg(ps, w, p1Tl, KO_1 // 2, t, wt1, off)
                    wt = min(w, V1 - off)
                    nc.scalar.activation(
                        exb[:, :wt], ps[:, :wt], Exp, scale=ISC,
                        accum_out=s1acc[:, t, si:si + 1])
                    if si < 3:
                        nc.vector.tensor_scalar_mul(
                            stg[:, off:off + wt], ps[:, :wt], ISC)
                    else:
                        nc.scalar.activation(
                            stg[:, off:off + wt], ps[:, :wt], Ident,
                            scale=ISC)
                nc.vector.reduce_sum(s1[:, t:t + 1], s1acc[:, t, :], axis=AX)

            def t1_ar(i):  # i = batch 0 (tiles 0-3) or 1 (tiles 4-7)
                nc.gpsimd.dma_start(cc1_in[i][:], s1[:, 4 * i:4 * i + 4])
                nc.gpsimd.collective_compute(
                    "AllReduce", mybir.AluOpType.add, replica_groups=rg,
                    ins=[cc1_in[i][:].opt()], outs=[cc1_out[i][:].opt()])

            def t1_bias(i):
                nc.gpsimd.dma_start(g1[:, 4 * i:4 * i + 4], cc1_out[i][:])
                lng = scratch.tile([P, 4], f32, tag="lng4")
                nc.scalar.activation(lng[:], g1[:, 4 * i:4 * i + 4], Ln)
                nc.vector.tensor_sub(out=b1[:, 4 * i:4 * i + 4],
                                     in0=cl[:, 4 * i:4 * i + 4, 0],
                                     in1=lng[:])

            def t1_finalize(t):
                nc.vector.tensor_scalar_add(stg1[t][:], stg1[t][:],
                                            b1[:, t:t + 1])
                nc.sync.dma_start(out_r[:, t, VH:VH + V1], stg1[t][:])

            for t in range(4):
                t1_compute(t)
            t1_ar(0)
            t1_compute(4)
            t1_compute(5)
            t1_compute(6)
            t1_bias(0)
            t1_finalize(0)
            t1_finalize(1)
            t1_compute(7)
            t1_ar(1)
            t1_finalize(2)
            t1_finalize(3)
            sp1E.release()
            wt1_pool.release()
            p1T_pool.release()

            # ================= Phase T2 (head tiles interleaved) ========
            headout_pool = tc.alloc_tile_pool(name="headoutp", bufs=3)
            sp2a = tc.alloc_tile_pool(name="sp2a", bufs=2)
            sp2b = [None]   # allocated after sp1L release
            obuf_pool = [None]
            stg2 = {}

            def t2_compute(t, mids=()):
                mids = dict(mids)
                pool = sp2a if (t // 2) % 2 == 0 else sp2b[0]
                stg = pool.tile([P, AW2], bf16, name=f"stg2_{t}", tag="s2")
                stg2[t] = stg
                for si, (off, w) in enumerate(T2SEGS):
                    for m in mids.get(si, ()):
                        m()
                    ps = psum.tile([P, SEG], f32, tag="mm")
                    mm_seg(ps, w, p2T, KO_2 // 2, t, wt2, off)
                    wt = min(w, V2 - off)
                    nc.scalar.activation(
                        exb[:, :wt], ps[:, :wt], Exp, scale=ISC,
                        accum_out=s2acc[:, t, si:si + 1])
                    if si < NA2:
                        nc.vector.tensor_scalar_mul(
                            stg[:, off:off + wt], ps[:, :wt], ISC)
                nc.vector.reduce_sum(s2[:, t:t + 1], s2acc[:, t, :], axis=AX)
                if t % 2:
                    b = t // 2
                    nc.gpsimd.dma_start(cc2_in[b][:], s2[:, t - 1:t + 1])
                    nc.gpsimd.collective_compute(
                        "AllReduce", mybir.AluOpType.add, replica_groups=rg,
                        ins=[cc2_in[b][:].opt()], outs=[cc2_out[b][:].opt()])

            def t2_bias(b):
                nc.gpsimd.dma_start(g2[:, 2 * b:2 * b + 2], cc2_out[b][:])
                lng = scratch.tile([P, 2], f32, tag="lng2")
                nc.scalar.activation(lng[:], g2[:, 2 * b:2 * b + 2], Ln)
                nc.vector.tensor_sub(out=b2[:, 2 * b:2 * b + 2],
                                     in0=cl[:, 2 * b:2 * b + 2, 1],
                                     in1=lng[:])



            def t2_finA(t):
                # staged region: add bias, ship
                nc.vector.tensor_scalar_add(stg2[t][:], stg2[t][:],
                                            b2[:, t:t + 1])
                nc.sync.dma_start(out_r[:, t, VH + V1:VH + V1 + AW2],
                                  stg2[t][:])

            obufs = {}

            def t2_defH(t, half):
                # deferred region: recompute, drain with fused bias
                if half == 0:
                    obufs[t] = obuf_pool[0].tile([P, BW2], bf16, tag="ob",
                                                 name=f"ob{t}")
                ob = obufs[t]
                for si in (0, 1) if half == 0 else (2, 3):
                    off, w = DEFSEGS[si]
                    ps = psum.tile([P, SEG], f32, tag="mm")
                    mm_seg(ps, w, p2T, KO_2 // 2, t, wt2, off)
                    wt = min(w, V2 - off)
                    o = ob[:, off - AW2:off - AW2 + wt]
                    if half == 0:
                        nc.scalar.activation(o, ps[:, :wt], Ident,
                                             scale=ISC, bias=b2[:, t:t + 1])
                    else:
                        nc.vector.tensor_scalar(
                            o, ps[:, :wt], ISC, b2[:, t:t + 1],
                            mybir.AluOpType.mult, mybir.AluOpType.add)
                if half == 1:
                    nc.sync.dma_start(out_r[:, t, VH + V1 + AW2:VOUT],
                                      ob[:, :BW2])

            # ============ Phase H (instruction-level interleave) ========
            # head subs run on their own 1-bank psum stream, spread into
            # the tail2 seg loop so the PE always has an independent
            # matmul to fill drain-bound gaps.
            hos = {}

            def h_sub(t, k):
                if k == 0:
                    hos[t] = headout_pool.tile([P, VHp], bf16, tag="ho",
                                               name=f"ho{t}")
                ho = hos[t]
                off, w = HSUBS[k]
                ps = psumS.tile([P, 512], f32, tag="hs")
                for kk in range(KO_H // 2):
                    nc.tensor.matmul(
                        ps[:, :w],
                        xT[:, 2 * kk:2 * kk + 2, t * P:(t + 1) * P],
                        whead[:, 2 * kk:2 * kk + 2, off:off + w],
                        start=(kk == 0), stop=(kk == KO_H // 2 - 1),
                        perf_mode=DR)
                if k < 4:
                    nc.vector.tensor_scalar_mul(
                        ho[:, off:off + w], ps[:, :w], ISC)
                else:
                    nc.scalar.activation(ho[:, off:off + w], ps[:, :w],
                                         Ident, scale=ISC)
                if k == 4:
                    nc.sync.dma_start(out_r[:, t, 0:VH], ho[:, :VH])

            def h_tile(t):
                for k in range(5):
                    h_sub(t, k)

            def h_mids(t):
                return {2: (lambda: h_sub(t, 0),),
                        4: (lambda: h_sub(t, 1),),
                        6: (lambda: h_sub(t, 2),),
                        8: (lambda: h_sub(t, 3),),
                        10: (lambda: h_sub(t, 4),)}

            def fin_mids(b):
                # finalize pair {2b, 2b+1} spread through tile 2b+3
                t0, t1x = 2 * b, 2 * b + 1
                return {3: (lambda: t2_bias(b), lambda: t2_finA(t0)),
                        5: (lambda: t2_defH(t0, 0),),
                        7: (lambda: t2_defH(t0, 1), lambda: t2_finA(t1x)),
                        9: (lambda: t2_defH(t1x, 0),),
                        11: (lambda: t2_defH(t1x, 1),)}

            h_tile(0)
            t2_compute(0, mids={5: (lambda: t1_bias(1),
                                    lambda: t1_finalize(4),
                                    lambda: t1_finalize(5),
                                    lambda: t1_finalize(6),
                                    lambda: t1_finalize(7))})
            sp1L.release()
            sp2b[0] = tc.alloc_tile_pool(name="sp2b", bufs=2)
            obuf_pool[0] = tc.alloc_tile_pool(name="obuf", bufs=2,
                                              side="right")
            t2_compute(1, mids=h_mids(1))
            t2_compute(2, mids=h_mids(2))
            t2_compute(3, mids={**h_mids(3), **fin_mids(0)})
            t2_compute(4, mids=h_mids(4))
            t2_compute(5, mids=fin_mids(1))
            t2_compute(6, mids=h_mids(5))
            t2_compute(7, mids=fin_mids(2))
            # tail: h6+h7 cover AR{6,7}; fin DMAs front-loaded, def
            # recomputes keep the PE fed while the out queue drains
            h_tile(6)
            h_tile(7)
            t2_bias(3)
            t2_finA(6)
            t2_defH(6, 0)
            t2_defH(6, 1)
            t2_finA(7)
            t2_defH(7, 0)
            t2_defH(7, 1)

            # unwind pools (stack order per side)
            sp2b[0].release()
            sp2a.release()
            headout_pool.release()
            obuf_pool[0].release()
            wt2_pool.release()
            whead_pool.release()
            xT_pool.release()

    nc.compile()
    return nc


def _get_nc():
    if "nc" not in _CACHE:
        _CACHE["nc"] = _build()
    return _CACHE["nc"]


def _prep_inputs(x, W_head, W_proj1, W_tail1, W_proj2, W_tail2):
    import concourse.mybir as mybir
    f8np = mybir.dt.np(mybir.dt.float8e4)

    def kxn8(w, pad_to=None, scale=WSC):
        # [N, K] weight -> [128, K//128, Np] fp8 (K on partitions), x scale
        n, k = w.shape
        a = np.ascontiguousarray(
            w.T.reshape(k // P, P, n).transpose(1, 0, 2)) * scale
        if pad_to is not None and pad_to != n:
            a = np.concatenate(
                [a, np.zeros((P, k // P, pad_to - n), np.float32)], axis=2)
        return a.astype(f8np)

    x2 = np.asarray(x, np.float32).reshape(T, H)
    xT = np.ascontiguousarray(
        x2.T.reshape(KO_H, P, T).transpose(1, 0, 2)).astype(f8np)
    wcl = kxn8(W_head[20000:20002])
    wp1 = kxn8(W_proj1)
    wp2 = kxn8(W_proj2)

    in_maps = []
    for i in range(N_CORES):
        in_maps.append({
            "xT": xT,
            "wheadT": kxn8(W_head[i * VH:(i + 1) * VH], VHp),
            "wclT": wcl,
            "wp1T": wp1,
            "wp2T": wp2,
            "wt1T": kxn8(W_tail1[i * V1:(i + 1) * V1], V1p),
            "wt2T": kxn8(W_tail2[i * V2:(i + 1) * V2], V2p),
        })
    return in_maps


def _assemble(outs):
    final = np.empty((T, 200000), dtype=np.float32)
    for i in range(N_CORES):
        o = np.asarray(outs[i]["out"])
        final[:, i * VH:(i + 1) * VH] = o[:, :VH]
        final[:, 20000 + i * V1:20000 + (i + 1) * V1] = o[:, VH:VH + V1]
        final[:, 60000 + i * V2:60000 + (i + 1) * V2] = o[:, VH + V1:]
    return final.reshape(2, 512, 200000)


def _run(inputs, trace=False, tmpdir=None):
    from concourse import bass_utils
    nc = _get_nc()
    in_maps = _prep_inputs(**inputs)
    res = bass_utils.run_bass_kernel_spmd(
        nc, in_maps, core_ids=list(range(N_CORES)), trace=trace,
        tmpdir=tmpdir)
    return _assemble(res.results), res


def kernel(**inputs):
    inputs = {k: np.asarray(v) for k, v in inputs.items()}
    out, _ = _run(inputs, trace=False)
    return out
